# revision 17
# baseline (speedup 1.0000x reference)
"""Trainium2 Bass kernel for the Tsit5 Neural-ODE problem.

Strategy (8 NeuronCores, data-parallel over batch):
  - B=2048 batch sharded 256/core; MLP params replicated; 199 sequential
    Tsit5 steps run locally per shard; no collectives.
  - Activations kept feature-major on chip: hidden h as [128, 2*256]
    (partition = feature tile, free = k-chunk*batch), y/r/acc as [64, 256]
    (partition = d, free = batch).
  - Matmuls in bf16 (PSUM fp32 accumulate). L1 bias is folded into the
    weight lhsT as two extra K rows (bf16 hi+lo) against constant ones rows
    kept in the rhs tiles; L2/L3 biases (BIAS_ACT=True) ride the Exp ops as
    per-partition fp32 bias vectors, one Exp per 128-feature m-chunk --
    this drops 48 PE instructions per step (24 rank-2 bias matmuls + their
    ldweights) and measures ~20% faster end-to-end than the bias-matmul
    form (11.1 ms vs 13.9 ms per 199-step trajectory, OUTER=201
    differential timing; the hot loop is instruction-fetch sensitive, so
    smaller bodies win -- UNROLL=2 measures 14% slower).
  - softplus(z) = Ln(Exp(z) + 1) -- both in the natural_log_exp_and_others
    ACT table set (bacc's set chooser is patched so no per-step table
    reloads happen); tanh(x) = 1 - 2*r with r = 1/(1 + e^{2x}) via the same
    Exp table + DVE reciprocal_approx_fast.
  - Tsit5 stage combinations accumulate directly from r_i with rescaled
    immediates (k_i = os*(1-2 r_i) is never materialized): running
    accumulators are seeded once per step and updated as each r_i lands,
    so only ONE fused scalar_tensor_tensor sits on the critical path per
    stage; that op also writes the bf16 rhs for the next stage directly.
  - Outputs: y_new transposed on PE (identity matmul) to batch-major,
    accumulated in two big SBUF staging buffers, flushed to DRAM after
    the time loop.
"""

import numpy as np
import ml_dtypes

B_, T_, D_, W_ = 2048, 200, 64, 256
NCORES = 8
BS = B_ // NCORES          # 256 batch per core
NSTEP = T_ - 1             # 199
LOOPN = None               # loop trip count override (timing experiments)
UNROLL = 1                 # steps per hardware-loop iteration
OUTER = 1                  # whole-trajectory repeats (timing experiments only)
BIAS_ACT = True            # True: hidden-layer biases via split-Exp bias vectors
                           # instead of rank-2 ones matmuls
HINT_PE = False            # arm PE branch-prefetch hints on the main loop
GP_OFF = False             # offload staging copies + acc seeds to GpSimd
STAGGER = True             # staggered_reset on the main loop

A21 = 0.161
A31, A32 = -0.008480655492356989, 0.335480655492357
A41, A42, A43 = 2.8971530571054935, -6.359448489975075, 4.3622954328695815
A51, A52, A53, A54 = 5.325864828439257, -11.748883564062828, 7.4955393428898365, -0.09249506636175525
A61, A62, A63, A64, A65 = 5.86145544294642, -12.92096931784711, 8.159367898576159, -0.071584973281401, -0.028269050394068383
B1c, B2c, B3c, B4c, B5c, B6c = (0.09646076681806523, 0.01, 0.4798896504144996,
                                1.379008574103742, -3.290069515436081, 2.324710524099774)

# STAGE_COEF[s] = coefficients of k_1..k_{s+1} for the argument of stage s+2
# (s = 0..4) and for y_new (s = 5).
STAGE_COEF = [
    [A21],
    [A31, A32],
    [A41, A42, A43],
    [A51, A52, A53, A54],
    [A61, A62, A63, A64, A65],
    [B1c, B2c, B3c, B4c, B5c, B6c],
]

_BUILD_CACHE = {}


def _patch_act_table_choice():
    """Make bacc's act-table chooser resolve Exp AND Ln to the one set that
    contains both (natural_log_exp_and_others) instead of each function's
    first-match set. Without this, every Exp<->Ln transition inserts an
    InstLoadActFuncSet (~36 table reloads per time step, ~70us each on HW).
    Set indices (= act_func_set_id) are preserved; we only hide Exp/Ln from
    the other sets so the CFG fixpoint hoists a single load out of the loop."""
    import concourse.bacc as bacc_mod
    import concourse.mybir as mybir
    if getattr(bacc_mod, "_nlx_act_patch", False):
        return
    AF = mybir.ActivationFunctionType
    orig = bacc_mod.get_activation_tables

    def patched(arch):
        tabs = orig(arch)
        out = {}
        for name, funcs in tabs.items():
            if name != "natural_log_exp_and_others":
                funcs = set(funcs) - {AF.Exp, AF.Ln}
            out[name] = funcs
        return out

    bacc_mod.get_activation_tables = patched
    bacc_mod._nlx_act_patch = True


def _build(dtc: float, out_scale: float):
    key = (float(dtc), float(out_scale), NSTEP, LOOPN, UNROLL, OUTER, BIAS_ACT,
           HINT_PE, GP_OFF, STAGGER)
    if key in _BUILD_CACHE:
        return _BUILD_CACHE[key]

    import concourse.mybir as mybir
    import concourse.tile as tile
    from concourse import bacc
    from concourse.bass import ds

    _patch_act_table_choice()

    dt = mybir.dt
    AF = mybir.ActivationFunctionType
    AO = mybir.AluOpType
    os_ = float(out_scale)

    nc = bacc.Bacc("TRN2", target_bir_lowering=False, debug=False)

    # ---- DRAM I/O ----
    y0t_d = nc.dram_tensor("y0t", [64, 256], dt.float32, kind="ExternalInput")
    w1t_d = nc.dram_tensor("w1t", [66, 256], dt.bfloat16, kind="ExternalInput")
    w2t_d = nc.dram_tensor("w2t", [128, 512], dt.bfloat16, kind="ExternalInput")
    w3t_d = nc.dram_tensor("w3t", [128, 512], dt.bfloat16, kind="ExternalInput")
    w4t_d = nc.dram_tensor("w4t", [128, 128], dt.bfloat16, kind="ExternalInput")
    bt2_d = nc.dram_tensor("bt2", [2, 256], dt.bfloat16, kind="ExternalInput")
    bt3_d = nc.dram_tensor("bt3", [2, 256], dt.bfloat16, kind="ExternalInput")
    ones2_d = nc.dram_tensor("ones2", [2, 256], dt.bfloat16, kind="ExternalInput")
    b4s_d = nc.dram_tensor("b4s", [64, 1], dt.float32, kind="ExternalInput")
    b2v_d = nc.dram_tensor("b2v", [128, 2], dt.float32, kind="ExternalInput")
    b3v_d = nc.dram_tensor("b3v", [128, 2], dt.float32, kind="ExternalInput")
    ident_d = nc.dram_tensor("ident", [64, 64], dt.float32, kind="ExternalInput")
    ys2_d = nc.dram_tensor("ys2", [2, 128, NSTEP * 64], dt.float32, kind="ExternalOutput")

    SC = NSTEP * 64  # staging columns

    # per-stage seed constants and update immediates
    # k_i = os*(1 - 2 r_i); arg_s = y + dtc * sum_i A[s][i] k_i
    seeds = [dtc * os_ * sum(c) for c in STAGE_COEF]            # 6 (args 2..6 + ynew)
    upd = [[-2.0 * dtc * os_ * c for c in row] for row in STAGE_COEF]

    loopn = NSTEP if LOOPN is None else LOOPN
    with tile.TileContext(nc) as tc:
        with (
            tc.tile_pool(name="const", bufs=1) as cp,
            tc.tile_pool(name="work", bufs=1) as wp,
            tc.tile_pool(name="stage", bufs=1) as sp_,
            tc.tile_pool(name="psum", bufs=1, space="PSUM") as pp,
        ):
            # constants
            w1t = cp.tile([66, 256], dt.bfloat16, tag="w1t")
            w2t = cp.tile([128, 512], dt.bfloat16, tag="w2t")
            w3t = cp.tile([128, 512], dt.bfloat16, tag="w3t")
            w4t = cp.tile([128, 128], dt.bfloat16, tag="w4t")
            bt2 = cp.tile([2, 256], dt.bfloat16, tag="bt2")
            bt3 = cp.tile([2, 256], dt.bfloat16, tag="bt3")
            ones2 = cp.tile([2, 256], dt.bfloat16, tag="ones2")
            b4s = cp.tile([64, 1], dt.float32, tag="b4s")
            b2v = cp.tile([128, 2], dt.float32, tag="b2v")
            b3v = cp.tile([128, 2], dt.float32, tag="b3v")
            ident = cp.tile([64, 64], dt.float32, tag="ident")
            loads = [(w1t, w1t_d), (w2t, w2t_d), (w3t, w3t_d), (w4t, w4t_d),
                     (b4s, b4s_d), (ident, ident_d)]
            if BIAS_ACT:
                loads += [(b2v, b2v_d), (b3v, b3v_d)]
            else:
                loads += [(bt2, bt2_d), (bt3, bt3_d), (ones2, ones2_d)]
            for t_, d_ in loads:
                nc.sync.dma_start(t_[:], d_[:])

            # state
            yf = wp.tile([64, 256], dt.float32, tag="yf")
            yb = wp.tile([66, 256], dt.bfloat16, tag="yb")
            args = [wp.tile([66, 256], dt.bfloat16, tag=f"arg{i}", name=f"arg{i}")
                    for i in range(5)]
            rs = [wp.tile([64, 256], dt.float32, tag=f"r{i}", name=f"r{i}")
                  for i in range(6)]
            acca = [wp.tile([64, 256], dt.float32, tag=f"acca{i}", name=f"acca{i}")
                    for i in range(5)]
            accy = wp.tile([64, 256], dt.float32, tag="accy")
            hs = [wp.tile([128, 512], dt.bfloat16, tag=f"h{i}", name=f"h{i}")
                  for i in range(3)]
            u_ = wp.tile([64, 256], dt.float32, tag="u")
            v_ = wp.tile([64, 256], dt.float32, tag="v")
            stage0 = sp_.tile([128, SC], dt.float32, tag="st0")
            stage1 = sp_.tile([128, SC], dt.float32, tag="st1")

            z1 = pp.tile([128, 512], dt.float32, tag="z1")
            z2 = pp.tile([128, 512], dt.float32, tag="z2")
            z3 = pp.tile([128, 512], dt.float32, tag="z3")
            z4 = pp.tile([64, 256], dt.float32, tag="z4")
            e_ = pp.tile([128, 512], dt.float32, tag="e")
            tp = pp.tile([128, 128], dt.float32, tag="tp")

            # ones rows of the bf16 rhs tiles (written once; the per-step
            # writes only touch rows 0:64)
            for tl in args + [yb]:
                nc.vector.memset(tl[64:66, :], 1.0)
            nc.sync.dma_start(yf[:], y0t_d[:])
            nc.vector.tensor_copy(yb[0:64, :], yf[:])

            def f_fwd(x_bf, s):
                """rs[s] = 1/(1 + exp(2*(W4 h3 + b4))) for x = arg of stage s."""
                # L1: z1 = [W1 ; b1_hi ; b1_lo]^T @ [x ; 1 ; 1], K=66
                for m in range(2):
                    cols = slice(m * 256, m * 256 + 256)
                    nc.tensor.matmul(z1[:, cols], w1t[:, m * 128:(m + 1) * 128],
                                     x_bf[:], start=True, stop=True)
                nc.scalar.activation(e_[:], z1[:], AF.Exp)
                nc.scalar.activation(hs[0][:], e_[:], AF.Ln, bias=1.0)
                # L2 / L3: K=256 in 2 chunks; bias via rank-1 matmul or Exp bias
                for li, (wt, bt, bv, hin, hout, zt) in enumerate(
                        [(w2t, bt2, b2v, hs[0], hs[1], z2),
                         (w3t, bt3, b3v, hs[1], hs[2], z3)]):
                    for m in range(2):
                        cols = slice(m * 256, m * 256 + 256)
                        if not BIAS_ACT:
                            nc.tensor.matmul(zt[:, cols], bt[:, m * 128:(m + 1) * 128],
                                             ones2[:], start=True, stop=False)
                        for c in range(2):
                            nc.tensor.matmul(zt[:, cols],
                                             wt[:, c * 256 + m * 128: c * 256 + m * 128 + 128],
                                             hin[:, c * 256:(c + 1) * 256],
                                             start=(BIAS_ACT and c == 0), stop=(c == 1))
                    if BIAS_ACT:
                        for m in range(2):
                            cols = slice(m * 256, m * 256 + 256)
                            nc.scalar.activation(e_[:, cols], zt[:, cols], AF.Exp,
                                                 bias=bv[:, m:m + 1])
                    else:
                        nc.scalar.activation(e_[:], zt[:], AF.Exp)
                    nc.scalar.activation(hout[:], e_[:], AF.Ln, bias=1.0)
                # L4: z4 [64, 256]
                for c in range(2):
                    nc.tensor.matmul(z4[:], w4t[:, c * 64:(c + 1) * 64],
                                     hs[2][:, c * 256:(c + 1) * 256],
                                     start=(c == 0), stop=(c == 1))
                # u = exp(2 z4 + 2 b4); r = 1/(1+u)
                nc.scalar.activation(u_[:], z4[:], AF.Exp, bias=b4s[:, 0:1], scale=2.0)
                nc.vector.tensor_scalar_add(v_[:], u_[:], 1.0)
                nc.vector.reciprocal_approx_fast(rs[s][:], v_[:])

            seed_eng = nc.gpsimd if GP_OFF else nc.vector

            def emit_step(toff):
                # seed the running accumulators (off critical path)
                for j in range(5):
                    seed_eng.tensor_scalar_add(acca[j][:], yf[:], seeds[j])
                seed_eng.tensor_scalar_add(accy[:], yf[:], seeds[5])

                x = yb
                for s in range(6):
                    f_fwd(x, s)
                    # critical-path: finish the next stage's argument (bf16)
                    if s < 5:
                        nc.vector.scalar_tensor_tensor(
                            args[s][0:64, :], rs[s][:], upd[s][s], acca[s][:],
                            AO.mult, AO.add)
                        x = args[s]
                    else:
                        nc.vector.scalar_tensor_tensor(
                            yf[:], rs[5][:], upd[5][5], accy[:], AO.mult, AO.add)
                    # off-critical updates of later accumulators
                    for j in range(s + 1, 5):
                        nc.vector.scalar_tensor_tensor(
                            acca[j][:], rs[s][:], upd[j][s], acca[j][:],
                            AO.mult, AO.add)
                    if s < 5:
                        nc.vector.scalar_tensor_tensor(
                            accy[:], rs[s][:], upd[5][s], accy[:],
                            AO.mult, AO.add)

                # outputs: next-step rhs + batch-major staging
                nc.vector.tensor_copy(yb[0:64, :], yf[:])
                nc.tensor.transpose(tp[:, 0:64], yf[:, 0:128], ident[:])
                nc.tensor.transpose(tp[:, 64:128], yf[:, 128:256], ident[:])
                st_eng = nc.vector  # GpSimd cannot read PSUM (tp)
                st_eng.tensor_copy(stage0[:, ds(toff, 64)], tp[:, 0:64])
                st_eng.tensor_copy(stage1[:, ds(toff, 64)], tp[:, 64:128])

            main_iters = loopn // UNROLL
            tail = loopn - main_iters * UNROLL
            hints = (mybir.EngineType.PE,) if (UNROLL > 1 or HINT_PE) else ()

            def emit_main_loop():
                with tc.For_i(0, main_iters, 1, staggered_reset=STAGGER,
                              hint_engines=hints) as t:
                    for u in range(UNROLL):
                        emit_step(t * (64 * UNROLL) + u * 64)

            if OUTER == 1:
                emit_main_loop()
            else:
                with tc.For_i(0, OUTER, 1):
                    emit_main_loop()
            for u in range(tail):
                emit_step((main_iters * UNROLL + u) * 64)

            nc.sync.dma_start(ys2_d[0], stage0[:])
            nc.sync.dma_start(ys2_d[1], stage1[:])

    nc.compile()
    _BUILD_CACHE[key] = nc
    return nc


def _prep_inputs(ts, y0, W1, b1, W2, b2, W3, b3, W4, b4, out_scale):
    bf = ml_dtypes.bfloat16
    ts = np.asarray(ts, np.float32)
    dtc = float(np.diff(ts.astype(np.float64)).mean())
    os_ = float(np.asarray(out_scale, np.float32))

    def hilo(b):
        b = np.asarray(b, np.float32)
        hi = b.astype(bf).astype(np.float32)
        lo = (b - hi).astype(bf)
        return hi.astype(bf), lo

    W1 = np.asarray(W1, np.float32)
    b1hi, b1lo = hilo(b1)
    w1t = np.empty((66, 256), bf)
    w1t[0:64] = np.ascontiguousarray(W1.T).astype(bf)
    w1t[64] = b1hi
    w1t[65] = b1lo

    def pack_w(Wm):  # [256,256] -> [128, 512]: (k, c*256 + m*128 + j) = W[m*128+j, c*128+k]
        Wm = np.asarray(Wm, np.float32)
        out = np.empty((128, 512), np.float32)
        for c in range(2):
            for m in range(2):
                out[:, c * 256 + m * 128: c * 256 + (m + 1) * 128] = \
                    Wm[m * 128:(m + 1) * 128, c * 128:(c + 1) * 128].T
        return out.astype(bf)

    w2t = pack_w(W2)
    w3t = pack_w(W3)
    w4 = np.asarray(W4, np.float32)
    w4t = np.empty((128, 128), np.float32)   # (k, c*64+j) = W4[j, c*128+k]
    for c in range(2):
        w4t[:, c * 64:(c + 1) * 64] = w4[:, c * 128:(c + 1) * 128].T
    w4t = w4t.astype(bf)

    bt2 = np.stack(hilo(b2), 0)
    bt3 = np.stack(hilo(b3), 0)
    ones2 = np.ones((2, 256), bf)
    b4s = (2.0 * np.asarray(b4, np.float32)).reshape(64, 1)
    b2v = np.stack([np.asarray(b2, np.float32)[0:128],
                    np.asarray(b2, np.float32)[128:256]], axis=1)
    b3v = np.stack([np.asarray(b3, np.float32)[0:128],
                    np.asarray(b3, np.float32)[128:256]], axis=1)
    ident = np.eye(64, dtype=np.float32)

    y0 = np.asarray(y0, np.float32)
    core_inputs = []
    for c in range(NCORES):
        sh = y0[c * BS:(c + 1) * BS]                     # [256, 64]
        core_inputs.append({
            "y0t": np.ascontiguousarray(sh.T, np.float32),   # [64, 256]
            "w1t": w1t, "w2t": w2t, "w3t": w3t, "w4t": w4t,
            "bt2": bt2, "bt3": bt3, "ones2": ones2,
            "b4s": np.ascontiguousarray(b4s, np.float32),
            "b2v": np.ascontiguousarray(b2v, np.float32),
            "b3v": np.ascontiguousarray(b3v, np.float32),
            "ident": ident,
        })
    return dtc, os_, core_inputs


def _run(trace=False, **inputs):
    from concourse.bass_utils import run_bass_kernel_spmd
    dtc, os_, core_inputs = _prep_inputs(**inputs)
    nc = _build(dtc, os_)
    res = run_bass_kernel_spmd(nc, core_inputs, core_ids=list(range(NCORES)),
                               trace=trace)
    y0 = np.asarray(inputs["y0"], np.float32)
    out = np.empty((B_, T_, D_), np.float32)
    out[:, 0, :] = y0
    for c in range(NCORES):
        ys2 = res.results[c]["ys2"]              # [2, 128, 199*64]
        out[c * BS: c * BS + 128, 1:, :] = ys2[0].reshape(128, NSTEP, 64)
        out[c * BS + 128:(c + 1) * BS, 1:, :] = ys2[1].reshape(128, NSTEP, 64)
    return out, res


def kernel(**inputs) -> np.ndarray:
    out, _ = _run(trace=False, **inputs)
    return out

